# revision 58
# baseline (speedup 1.0000x reference)
"""Single-head causal attention for Trainium2, batch-parallel over 8 NeuronCores.

Reference computation (per batch element b):
    q = x @ Wq + bq; k = x @ Wk + bk; v = x @ Wv + bv        # [T, H]
    s = q @ k.T / sqrt(H); causal mask; w = softmax(s)
    out = w @ v                                              # [T, H]

Shapes: x [8, 2048, 1024] f32, W* [1024, 128], b* [128]. Output [8, 2048, 128].

Strategy: one batch element per core (pure data parallel, no collectives).
The data path runs in bf16 (inputs converted host-side; measured rel err
~4e-3 vs the 2e-2 gate):

  - x, W (pre-transposed) and the constants ship as bf16 and land in SBUF
    via XBAR DMA-transpose loads - no PE transposes, no PSUM staging, no
    drain copies. All loads are the same instruction type on one queue:
    mixing DMACopy/DmaTransposeAnt (or queues) makes the tile scheduler
    serialize each pair head-to-tail.
  - Q.T/K.T [H, T] = W.T @ x.T (contract E on partitions). V is computed
    directly in [t, h] blocks (lhsT = x.T chunk slice, rhs = Wv chunk), so
    the AV matmul needs no V transpose; V's bias is a broadcast matrix
    built with one 1-partition matmul.
  - Scores are computed transposed, S.T[k, q] = (K.T chunk).T @ Q.T, only
    over the causal lower triangle at 128x512 blocks; exp on ACT (scale
    fused), one triangular mask multiply on the diagonal (DVE).
  - AV is computed directly in [q, h]: lhsT = P.T column-slice (q-tile),
    rhs = V block extended with a ones column, so output column 128 of
    each q-tile accumulates the softmax row sums in the exact layout the
    normalization needs: rinv = 1/psum-column, one tensor_scalar, store.
  - The attention groups are ACT(exp)-throughput-bound, so the panel-2/3
    Q/K projections are interleaved into attention groups 0-1 and the
    panel-2/3 V projections into groups 2-3, keeping the tensor engine
    busy inside the exp-latency bubbles. Scores prefetch 3 deep so the
    exp stream never waits. The last group defers its normalizations past
    its final AV matmul (a scale reading the OT psum tile would stall the
    next k-block's accumulate), batching reciprocals ahead of scales.
"""

import sys

if "/opt/trn_rl_repo" not in sys.path:
    sys.path.insert(0, "/opt/trn_rl_repo")

import numpy as np
import ml_dtypes

import concourse.bacc as bacc
import concourse.mybir as mybir
import concourse.tile as tile
from concourse.bass_utils import run_bass_kernel_spmd

F32 = mybir.dt.float32
BF16 = mybir.dt.bfloat16
AF = mybir.ActivationFunctionType

B, T, E, H = 8, 2048, 1024, 128
NE = E // 128  # 8 e-chunks
NT = T // 128  # 16 t-tiles
NG = T // 512  # 4 q-groups
SCALE = 1.0 / float(np.sqrt(H))
CBC = 272  # constants tile columns (DRAM rows; padded to a 16 multiple)


def _emit(nc, tc, x, wbt, cbt_d, out):
    with (
        tc.tile_pool(name="const", bufs=1) as cpool,
        tc.tile_pool(name="wpool", bufs=1) as wpool,
        tc.tile_pool(name="pers", bufs=1) as pers,
        tc.tile_pool(name="ptp", bufs=1) as ptp,
        tc.tile_pool(name="small", bufs=1) as smallp,
        tc.tile_pool(name="psum", bufs=1, space="PSUM") as psp,
    ):
        cbt = cpool.tile([128, CBC], BF16)
        tri = cbt[:, 1:129]           # upper-tri incl diag (keep k<=q in [k,q])
        ones_row = cbt[0:1, 1:129]    # tri row 0 is all ones
        bv_row = cbt[0:1, 129:257]    # bv in partition 0

        wt = wpool.tile([128, NE * 3 * H], BF16)
        XT = [pers.tile([128, T], BF16, tag=f"xt{e}", name=f"xt{e}") for e in range(NE)]

        # One queue, one instruction type, need-order: constants, then
        # (w chunk, x half) pairs for panels 0/1, then the panel-2/3 halves.
        nc.sync.dma_start_transpose(cbt[:], cbt_d[:, :])
        for e in range(NE):
            nc.sync.dma_start_transpose(
                XT[e][:, 0:1024], x[0:1024, e * 128 : (e + 1) * 128]
            )
            nc.sync.dma_start_transpose(
                wt[:, e * 384 : (e + 1) * 384], wbt[:, e * 128 : (e + 1) * 128]
            )
        for e in range(NE):
            nc.sync.dma_start_transpose(
                XT[e][:, 1024:2048], x[1024:2048, e * 128 : (e + 1) * 128]
            )

        def w_chunk(eb, i):
            return wt[:, eb * 3 * H + i * H : eb * 3 * H + (i + 1) * H]

        # tensor_scalar wants f32 scalars; upconvert the bf16 bias columns
        # once on the (idle) gpsimd engine. (Emitted late: PE executes in
        # order, so constant-dependent work must not head the PE queue.)
        BQK = smallp.tile([128, 2], F32, tag="bqk")
        bq_col = BQK[:, 0:1]
        bk_col = BQK[:, 1:2]

        QT = pers.tile([128, T], BF16, tag="qt")
        KT = pers.tile([128, T], BF16, tag="kt")
        # V blocks [k, h | 1] at cols 129*kblk: col 128 of each block is a
        # ones column so the AV matmul also emits softmax row sums.
        VNx = pers.tile([128, NT * 129], BF16, tag="vn")
        nc.vector.memset(
            VNx[:].rearrange("p (b c) -> p b c", c=129)[:, :, 128:129], 1.0
        )
        BVs = smallp.tile([128, 128], F32, tag="bvs")

        def qk_ebs(panels, tag):
            """Q/K accumulation e-chunk steps for `panels`; returns (step_fn,
            drain_fn) where step_fn(eb) emits that chunk's matmuls."""
            acc = {
                p: (
                    psp.tile([128, 512], F32, tag=tag, bufs=4, name=f"ppq{p}"),
                    psp.tile([128, 512], F32, tag=tag, bufs=4, name=f"ppk{p}"),
                )
                for p in panels
            }

            def step(eb, subset=None):
                st, sp = eb == 0, eb == NE - 1
                for p in (subset or panels):
                    cols = slice(p * 512, (p + 1) * 512)
                    ppq, ppk = acc[p]
                    nc.tensor.matmul(ppq[:], lhsT=w_chunk(eb, 0),
                                     rhs=XT[eb][:, cols], start=st, stop=sp)
                    nc.tensor.matmul(ppk[:], lhsT=w_chunk(eb, 1),
                                     rhs=XT[eb][:, cols], start=st, stop=sp)

            def drain():
                for p in panels:
                    cols = slice(p * 512, (p + 1) * 512)
                    ppq, ppk = acc[p]
                    nc.vector.tensor_scalar_add(QT[:, cols], ppq[:], bq_col)
                    nc.vector.tensor_scalar_add(KT[:, cols], ppk[:], bk_col)

            return step, drain

        def v_ebs(p):
            """V accumulation steps for panel p ([t,h] blocks, 4 q-tiles in
            one psum bank as column slices)."""
            vacc = psp.tile([128, 512], F32, tag="vac", bufs=2, name=f"vacc{p}")

            def step(eb):
                st, sp = eb == 0, eb == NE - 1
                for ti in range(4):
                    tsl = slice(p * 512 + ti * 128, p * 512 + (ti + 1) * 128)
                    # start marks the whole 2KB bank pending-zero; later
                    # slices write-first into still-pending bytes.
                    nc.tensor.matmul(
                        vacc[:, ti * 128 : (ti + 1) * 128],
                        lhsT=XT[eb][:, tsl], rhs=w_chunk(eb, 2),
                        start=(st and ti == 0), stop=(sp and ti == 3),
                        skip_group_check=True,
                    )

            def drain():
                for ti in range(4):
                    base = (p * 4 + ti) * 129
                    nc.vector.tensor_add(
                        VNx[:, base : base + 128],
                        vacc[:, ti * 128 : (ti + 1) * 128], BVs[:]
                    )

            return step, drain

        # PSUM q-tile offsets inside the 2-bank [128,1024] OT tile: each
        # [q,129] slice must not cross a 2KB bank boundary.
        OFFS = (0, 129, 258, 512)

        def attn_group(g, s_tag="qk", s_bufs=4, defer_retire=False):
            """Attention q-group g as three emitters: sx(k) score matmul,
            ex(k) exp+mask, av(k) AV matmuls (+retire at each q-tile stop)."""
            qlo = g * 512
            nk = 4 * g + 4
            pso = psp.tile([128, 1024], F32, tag="ot", bufs=1, name=f"ot{g}")
            last_g = g == NG - 1
            obg = smallp.tile([128, 512], F32, tag="obg", bufs=2, name=f"obg{g}")
            psts = {}
            pts = {}

            def sx(kblk):
                if kblk >= nk:
                    return
                j = kblk - 4 * g
                off = max(j, 0) * 128
                pst = psp.tile([128, 512], F32, tag=s_tag, bufs=s_bufs,
                               name=f"st{g}_{kblk}")
                nc.tensor.matmul(
                    pst[:, off:],
                    lhsT=KT[:, kblk * 128 : (kblk + 1) * 128],
                    rhs=QT[:, qlo + off : qlo + 512],
                    start=True, stop=True,
                )
                psts[kblk] = pst

            def ex(kblk):
                j = kblk - 4 * g
                off = max(j, 0) * 128
                pt = ptp.tile([128, 512], BF16, tag=f"pt{g % 2}_{kblk}",
                              name=f"pt{g}_{kblk}")
                nc.scalar.activation(pt[:, off:], psts.pop(kblk)[:, off:],
                                     AF.Exp, scale=SCALE)
                if j >= 0:
                    dsl = slice(j * 128, (j + 1) * 128)
                    nc.vector.tensor_mul(pt[:, dsl], pt[:, dsl], tri)
                pts[kblk] = pt

            rinvs = {}

            def rinv_of(qt):
                rinv = smallp.tile([128, 1], F32, tag="rinv", bufs=4,
                                   name=f"rinv{g}_{qt}")
                nc.vector.reciprocal(
                    rinv[:], pso[:, OFFS[qt] + 128 : OFFS[qt] + 129]
                )
                rinvs[qt] = rinv

            def retire(qt):
                """scale + store once a q-tile's accumulation has stopped."""
                if qt not in rinvs:
                    rinv_of(qt)
                nc.vector.tensor_scalar_mul(
                    obg[:, qt * 128 : (qt + 1) * 128],
                    pso[:, OFFS[qt] : OFFS[qt] + 128], rinvs[qt][:]
                )
                if last_g and qt == 2:
                    nc.sync.dma_start(
                        out[qlo : qlo + 384, :].rearrange(
                            "(qt p) h -> p qt h", p=128
                        ),
                        obg[:, 0:384].rearrange("p (qt h) -> p qt h", h=H),
                    )
                elif last_g and qt == 3:
                    nc.sync.dma_start(
                        out[qlo + 384 : qlo + 512, :], obg[:, 384:512]
                    )
                elif qt == 3:
                    nc.sync.dma_start(
                        out[qlo : qlo + 512, :].rearrange(
                            "(qt p) h -> p qt h", p=128
                        ),
                        obg[:].rearrange("p (qt h) -> p qt h", h=H),
                    )

            def av(kblk):
                j = kblk - 4 * g
                pt = pts[kblk]
                for qt in range(4):
                    if j > qt:
                        continue
                    nc.tensor.matmul(
                        pso[:, OFFS[qt] : OFFS[qt] + 129],
                        lhsT=pt[:, qt * 128 : (qt + 1) * 128],
                        rhs=VNx[:, kblk * 129 : (kblk + 1) * 129],
                        start=(kblk == 0 and qt in (0, 3)),
                        stop=(kblk == 4 * g + qt),
                        skip_group_check=True,
                    )
                    if kblk == 4 * g + qt and not defer_retire:
                        retire(qt)

            def finish():
                if defer_retire:
                    # all reciprocals first (independent), then scales with
                    # qt3 first so the tail store's DMA pipe starts earliest
                    for qt in range(4):
                        rinv_of(qt)
                    for qt in (0, 1, 3, 2):
                        retire(qt)

            return sx, ex, av, nk, finish

        # ---- schedule ----
        # proj 0/1: Q,K,V for panels 0 and 1, e-chunk-major (DMA-paced).
        qk01_step, qk01_drain = qk_ebs((0, 1), "qk")
        v0_step, v0_drain = v_ebs(0)
        v1_step, v1_drain = v_ebs(1)
        for eb in range(NE):
            qk01_step(eb, (0,))
            v0_step(eb)
            qk01_step(eb, (1,))
            v1_step(eb)
        # V-bias broadcast matrix: BV[i,j] = bv[j] via 1-partition matmul.
        nc.gpsimd.tensor_copy(BQK[:], cbt[:, 257:259])
        bvps = psp.tile([128, 128], F32, tag="ot", bufs=1, name="bvps")
        nc.tensor.matmul(bvps[:], lhsT=ones_row, rhs=bv_row, start=True, stop=True)
        nc.scalar.copy(BVs[:], bvps[:])
        qk01_drain()
        v0_drain()
        v1_drain()

        # B0 and B1 interleave the Q/K projections for panels 2,3:
        # attention is exp(ACT)-throughput-bound, projections are pure PE,
        # so the mix keeps both engines fed. The first two e-chunk steps go
        # into B0 (their x halves have landed by then).
        qk23_step, qk23_drain = qk_ebs((2, 3), "qk")
        sx0, ex0, av0, nk0, fin0 = attn_group(0, s_tag="vac", s_bufs=2)
        sx0(0)
        for k in range(nk0):
            sx0(k + 1)
            ex0(k)
            av0(k)
            if k >= 2:
                qk23_step(k - 2)

        sx1, ex1, av1, nk1, fin1 = attn_group(1, s_tag="vac", s_bufs=2)
        sx1(0)
        for k in range(nk1):
            sx1(k + 1)
            ex1(k)
            av1(k)
            if k < 6:
                qk23_step(k + 2)
        qk23_drain()

        # B2 with the panel-2 V projection interleaved, B3 with panel-3's:
        # keeps each attention group just-barely ACT-bound instead of
        # making B2 PE-bound while B3's tensor engine starves.
        v2_step, v2_drain = v_ebs(2)
        v3_step, v3_drain = v_ebs(3)

        def run_group(g, sx, ex, av, nk, vsched):
            sx(0)
            sx(1)
            sx(2)
            for k in range(nk):
                sx(k + 3)
                ex(k)
                step = vsched.get(k)
                if step is not None:
                    fn, arg = step
                    if fn is None:
                        arg()
                    else:
                        for eb in arg:
                            fn(eb)
                av(k)

        v2_sched = {0: (v2_step, (0, 1)), 1: (v2_step, (2, 3)),
                    2: (v2_step, (4, 5)), 3: (v2_step, (6, 7)),
                    4: (None, v2_drain)}
        sx2, ex2, av2, nk2, fin2 = attn_group(2)
        run_group(2, sx2, ex2, av2, nk2, v2_sched)

        v3_sched = {0: (v3_step, (0, 1)), 1: (v3_step, (2, 3)),
                    2: (v3_step, (4, 5)), 3: (v3_step, (6, 7)),
                    4: (None, v3_drain)}
        sx3, ex3, av3, nk3, fin3 = attn_group(3, defer_retire=True)
        run_group(3, sx3, ex3, av3, nk3, v3_sched)
        fin3()


def build_program():
    nc = bacc.Bacc("TRN2", target_bir_lowering=False, debug=False)
    x = nc.dram_tensor("x", [T, E], BF16, kind="ExternalInput").ap()
    wbt = nc.dram_tensor("wbt", [3 * H, E], BF16, kind="ExternalInput").ap()
    cbt_d = nc.dram_tensor("cbt_d", [CBC, 128], BF16, kind="ExternalInput").ap()
    out = nc.dram_tensor("out", [T, H], F32, kind="ExternalOutput").ap()
    with tile.TileContext(nc) as tc:
        _emit(nc, tc, x, wbt, cbt_d, out)
    nc.compile()
    return nc


_program = None


def _get_program():
    global _program
    if _program is None:
        _program = build_program()
    return _program


def make_in_maps(x, Wq, Wk, Wv, bq, bk, bv):
    bf = ml_dtypes.bfloat16
    x = np.asarray(x, np.float32).astype(bf)
    wall = np.concatenate(
        [np.asarray(w, np.float32) for w in (Wq, Wk, Wv)], axis=1
    ).astype(bf)
    wbt = np.ascontiguousarray(wall.T)
    cb = np.zeros((128, CBC), dtype=bf)
    cb[:, 0] = bf(1.0)
    cb[:, 1:129] = np.triu(np.ones((128, 128), np.float32)).astype(bf)
    cb[0, 129:257] = np.asarray(bv, np.float32).reshape(H).astype(bf)
    cb[:, 257] = np.asarray(bq, np.float32).reshape(H).astype(bf)
    cb[:, 258] = np.asarray(bk, np.float32).reshape(H).astype(bf)
    cbt_d = np.ascontiguousarray(cb.T)
    return [
        {"x": np.ascontiguousarray(x[b]), "wbt": wbt, "cbt_d": cbt_d}
        for b in range(B)
    ]


def kernel(x, Wq, Wk, Wv, bq, bk, bv):
    nc = _get_program()
    in_maps = make_in_maps(x, Wq, Wk, Wv, bq, bk, bv)
    res = run_bass_kernel_spmd(nc, in_maps, list(range(B)))
    return np.stack([res.results[b]["out"] for b in range(B)], axis=0).astype(
        np.float32
    )


# revision 59
# speedup vs baseline: 1.0075x; 1.0075x over previous
"""Single-head causal attention for Trainium2, batch-parallel over 8 NeuronCores.

Reference computation (per batch element b):
    q = x @ Wq + bq; k = x @ Wk + bk; v = x @ Wv + bv        # [T, H]
    s = q @ k.T / sqrt(H); causal mask; w = softmax(s)
    out = w @ v                                              # [T, H]

Shapes: x [8, 2048, 1024] f32, W* [1024, 128], b* [128]. Output [8, 2048, 128].

Strategy: one batch element per core (pure data parallel, no collectives).
The data path runs in bf16 (inputs converted host-side; measured rel err
~4e-3 vs the 2e-2 gate):

  - x, W (pre-transposed) and the constants ship as bf16 and land in SBUF
    via XBAR DMA-transpose loads - no PE transposes, no PSUM staging, no
    drain copies. All loads are the same instruction type on one queue:
    mixing DMACopy/DmaTransposeAnt (or queues) makes the tile scheduler
    serialize each pair head-to-tail.
  - Q.T/K.T [H, T] = W.T @ x.T (contract E on partitions). V is computed
    directly in [t, h] blocks (lhsT = x.T chunk slice, rhs = Wv chunk), so
    the AV matmul needs no V transpose; V's bias is a broadcast matrix
    built with one 1-partition matmul.
  - Scores are computed transposed, S.T[k, q] = (K.T chunk).T @ Q.T, only
    over the causal lower triangle at 128x512 blocks; exp on ACT (scale
    fused), one triangular mask multiply on the diagonal (DVE).
  - AV is computed directly in [q, h]: lhsT = P.T column-slice (q-tile),
    rhs = V block extended with a ones column, so output column 128 of
    each q-tile accumulates the softmax row sums in the exact layout the
    normalization needs: rinv = 1/psum-column, one tensor_scalar, store.
  - The attention groups are ACT(exp)-throughput-bound, so the panel-2/3
    Q/K projections are interleaved into attention groups 0-1 and the
    panel-2/3 V projections into groups 2-3, keeping the tensor engine
    busy inside the exp-latency bubbles. Scores prefetch 3 deep so the
    exp stream never waits. The last group defers its normalizations past
    its final AV matmul (a scale reading the OT psum tile would stall the
    next k-block's accumulate), batching reciprocals ahead of scales.
"""

import sys

if "/opt/trn_rl_repo" not in sys.path:
    sys.path.insert(0, "/opt/trn_rl_repo")

import numpy as np
import ml_dtypes

import concourse.bacc as bacc
import concourse.mybir as mybir
import concourse.tile as tile
from concourse.bass_utils import run_bass_kernel_spmd

F32 = mybir.dt.float32
BF16 = mybir.dt.bfloat16
AF = mybir.ActivationFunctionType

B, T, E, H = 8, 2048, 1024, 128
NE = E // 128  # 8 e-chunks
NT = T // 128  # 16 t-tiles
NG = T // 512  # 4 q-groups
SCALE = 1.0 / float(np.sqrt(H))
CBC = 272  # constants tile columns (DRAM rows; padded to a 16 multiple)


def _emit(nc, tc, x, wbt, cbt_d, out):
    with (
        tc.tile_pool(name="const", bufs=1) as cpool,
        tc.tile_pool(name="wpool", bufs=1) as wpool,
        tc.tile_pool(name="pers", bufs=1) as pers,
        tc.tile_pool(name="ptp", bufs=1) as ptp,
        tc.tile_pool(name="small", bufs=1) as smallp,
        tc.tile_pool(name="psum", bufs=1, space="PSUM") as psp,
    ):
        cbt = cpool.tile([128, CBC], BF16)
        tri = cbt[:, 1:129]           # upper-tri incl diag (keep k<=q in [k,q])
        ones_row = cbt[0:1, 1:129]    # tri row 0 is all ones
        bv_row = cbt[0:1, 129:257]    # bv in partition 0

        wt = wpool.tile([128, NE * 3 * H], BF16)
        XT = [pers.tile([128, T], BF16, tag=f"xt{e}", name=f"xt{e}") for e in range(NE)]

        # One queue, one instruction type, need-order: constants, then
        # (w chunk, x half) pairs for panels 0/1, then the panel-2/3 halves.
        nc.sync.dma_start_transpose(cbt[:], cbt_d[:, :])
        for e in range(NE):
            nc.sync.dma_start_transpose(
                XT[e][:, 0:1024], x[0:1024, e * 128 : (e + 1) * 128]
            )
            nc.sync.dma_start_transpose(
                wt[:, e * 384 : (e + 1) * 384], wbt[:, e * 128 : (e + 1) * 128]
            )
        for e in range(NE):
            nc.sync.dma_start_transpose(
                XT[e][:, 1024:2048], x[1024:2048, e * 128 : (e + 1) * 128]
            )

        def w_chunk(eb, i):
            return wt[:, eb * 3 * H + i * H : eb * 3 * H + (i + 1) * H]

        # tensor_scalar wants f32 scalars; upconvert the bf16 bias columns
        # once on the (idle) gpsimd engine. (Emitted late: PE executes in
        # order, so constant-dependent work must not head the PE queue.)
        BQK = smallp.tile([128, 2], F32, tag="bqk")
        bq_col = BQK[:, 0:1]
        bk_col = BQK[:, 1:2]

        QT = pers.tile([128, T], BF16, tag="qt")
        KT = pers.tile([128, T], BF16, tag="kt")
        # V blocks [k, h | 1] at cols 129*kblk: col 128 of each block is a
        # ones column so the AV matmul also emits softmax row sums.
        VNx = pers.tile([128, NT * 129], BF16, tag="vn")
        nc.vector.memset(
            VNx[:].rearrange("p (b c) -> p b c", c=129)[:, :, 128:129], 1.0
        )
        BVs = smallp.tile([128, 128], F32, tag="bvs")

        def qk_ebs(panels, tag):
            """Q/K accumulation e-chunk steps for `panels`; returns (step_fn,
            drain_fn) where step_fn(eb) emits that chunk's matmuls."""
            acc = {
                p: (
                    psp.tile([128, 512], F32, tag=tag, bufs=4, name=f"ppq{p}"),
                    psp.tile([128, 512], F32, tag=tag, bufs=4, name=f"ppk{p}"),
                )
                for p in panels
            }

            def step(eb, subset=None):
                st, sp = eb == 0, eb == NE - 1
                for p in (subset or panels):
                    cols = slice(p * 512, (p + 1) * 512)
                    ppq, ppk = acc[p]
                    nc.tensor.matmul(ppq[:], lhsT=w_chunk(eb, 0),
                                     rhs=XT[eb][:, cols], start=st, stop=sp)
                    nc.tensor.matmul(ppk[:], lhsT=w_chunk(eb, 1),
                                     rhs=XT[eb][:, cols], start=st, stop=sp)

            def drain():
                for p in panels:
                    cols = slice(p * 512, (p + 1) * 512)
                    ppq, ppk = acc[p]
                    nc.vector.tensor_scalar_add(QT[:, cols], ppq[:], bq_col)
                    nc.vector.tensor_scalar_add(KT[:, cols], ppk[:], bk_col)

            return step, drain

        def v_ebs(p):
            """V accumulation steps for panel p ([t,h] blocks, 4 q-tiles in
            one psum bank as column slices)."""
            vacc = psp.tile([128, 512], F32, tag="vac", bufs=2, name=f"vacc{p}")

            def step(eb):
                st, sp = eb == 0, eb == NE - 1
                for ti in range(4):
                    tsl = slice(p * 512 + ti * 128, p * 512 + (ti + 1) * 128)
                    # start marks the whole 2KB bank pending-zero; later
                    # slices write-first into still-pending bytes.
                    nc.tensor.matmul(
                        vacc[:, ti * 128 : (ti + 1) * 128],
                        lhsT=XT[eb][:, tsl], rhs=w_chunk(eb, 2),
                        start=(st and ti == 0), stop=(sp and ti == 3),
                        skip_group_check=True,
                    )

            def drain():
                for ti in range(4):
                    base = (p * 4 + ti) * 129
                    nc.vector.tensor_add(
                        VNx[:, base : base + 128],
                        vacc[:, ti * 128 : (ti + 1) * 128], BVs[:]
                    )

            return step, drain

        # PSUM q-tile offsets inside the 2-bank [128,1024] OT tile: each
        # [q,129] slice must not cross a 2KB bank boundary.
        OFFS = (0, 129, 258, 512)

        def attn_group(g, s_tag="qk", s_bufs=4, defer_retire=False):
            """Attention q-group g as three emitters: sx(k) score matmul,
            ex(k) exp+mask, av(k) AV matmuls (+retire at each q-tile stop)."""
            qlo = g * 512
            nk = 4 * g + 4
            pso = psp.tile([128, 1024], F32, tag="ot", bufs=1, name=f"ot{g}")
            last_g = g == NG - 1
            obg = smallp.tile([128, 512], F32, tag="obg", bufs=2, name=f"obg{g}")
            psts = {}
            pts = {}

            def sx(kblk):
                if kblk >= nk:
                    return
                j = kblk - 4 * g
                off = max(j, 0) * 128
                pst = psp.tile([128, 512], F32, tag=s_tag, bufs=s_bufs,
                               name=f"st{g}_{kblk}")
                nc.tensor.matmul(
                    pst[:, off:],
                    lhsT=KT[:, kblk * 128 : (kblk + 1) * 128],
                    rhs=QT[:, qlo + off : qlo + 512],
                    start=True, stop=True,
                )
                psts[kblk] = pst

            def ex(kblk):
                j = kblk - 4 * g
                off = max(j, 0) * 128
                pt = ptp.tile([128, 512], BF16, tag=f"pt{g % 2}_{kblk}",
                              name=f"pt{g}_{kblk}")
                nc.scalar.activation(pt[:, off:], psts.pop(kblk)[:, off:],
                                     AF.Exp, scale=SCALE)
                if j >= 0:
                    dsl = slice(j * 128, (j + 1) * 128)
                    nc.vector.tensor_mul(pt[:, dsl], pt[:, dsl], tri)
                pts[kblk] = pt

            rinvs = {}

            def rinv_of(qt):
                rinv = smallp.tile([128, 1], F32, tag="rinv", bufs=4,
                                   name=f"rinv{g}_{qt}")
                nc.vector.reciprocal(
                    rinv[:], pso[:, OFFS[qt] + 128 : OFFS[qt] + 129]
                )
                rinvs[qt] = rinv

            def retire(qt):
                """scale + store once a q-tile's accumulation has stopped."""
                if qt not in rinvs:
                    rinv_of(qt)
                nc.vector.tensor_scalar_mul(
                    obg[:, qt * 128 : (qt + 1) * 128],
                    pso[:, OFFS[qt] : OFFS[qt] + 128], rinvs[qt][:]
                )
                if last_g and qt == 2:
                    nc.sync.dma_start(
                        out[qlo : qlo + 384, :].rearrange(
                            "(qt p) h -> p qt h", p=128
                        ),
                        obg[:, 0:384].rearrange("p (qt h) -> p qt h", h=H),
                    )
                elif last_g and qt == 3:
                    nc.sync.dma_start(
                        out[qlo + 384 : qlo + 512, :], obg[:, 384:512]
                    )
                elif qt == 3:
                    nc.sync.dma_start(
                        out[qlo : qlo + 512, :].rearrange(
                            "(qt p) h -> p qt h", p=128
                        ),
                        obg[:].rearrange("p (qt h) -> p qt h", h=H),
                    )

            def av(kblk):
                j = kblk - 4 * g
                pt = pts[kblk]
                for qt in range(4):
                    if j > qt:
                        continue
                    nc.tensor.matmul(
                        pso[:, OFFS[qt] : OFFS[qt] + 129],
                        lhsT=pt[:, qt * 128 : (qt + 1) * 128],
                        rhs=VNx[:, kblk * 129 : (kblk + 1) * 129],
                        start=(kblk == 0 and qt in (0, 3)),
                        stop=(kblk == 4 * g + qt),
                        skip_group_check=True,
                    )
                    if kblk == 4 * g + qt and not defer_retire:
                        retire(qt)

            def finish():
                if defer_retire:
                    # all reciprocals first (independent), then scales with
                    # qt3 first so the tail store's DMA pipe starts earliest
                    for qt in range(4):
                        rinv_of(qt)
                    for qt in range(4):
                        retire(qt)

            return sx, ex, av, nk, finish

        # ---- schedule ----
        # proj 0/1: Q,K,V for panels 0 and 1, e-chunk-major (DMA-paced).
        qk01_step, qk01_drain = qk_ebs((0, 1), "qk")
        v0_step, v0_drain = v_ebs(0)
        v1_step, v1_drain = v_ebs(1)
        for eb in range(NE):
            qk01_step(eb, (0,))
            v0_step(eb)
            qk01_step(eb, (1,))
            v1_step(eb)
        # V-bias broadcast matrix: BV[i,j] = bv[j] via 1-partition matmul.
        nc.gpsimd.tensor_copy(BQK[:], cbt[:, 257:259])
        bvps = psp.tile([128, 128], F32, tag="ot", bufs=1, name="bvps")
        nc.tensor.matmul(bvps[:], lhsT=ones_row, rhs=bv_row, start=True, stop=True)
        nc.scalar.copy(BVs[:], bvps[:])
        qk01_drain()
        v0_drain()
        v1_drain()

        # B0 and B1 interleave the Q/K projections for panels 2,3:
        # attention is exp(ACT)-throughput-bound, projections are pure PE,
        # so the mix keeps both engines fed. The first two e-chunk steps go
        # into B0 (their x halves have landed by then).
        qk23_step, qk23_drain = qk_ebs((2, 3), "qk")
        sx0, ex0, av0, nk0, fin0 = attn_group(0, s_tag="vac", s_bufs=2)
        sx0(0)
        for k in range(nk0):
            sx0(k + 1)
            ex0(k)
            av0(k)
            if k >= 2:
                qk23_step(k - 2)

        sx1, ex1, av1, nk1, fin1 = attn_group(1, s_tag="vac", s_bufs=2)
        sx1(0)
        for k in range(nk1):
            sx1(k + 1)
            ex1(k)
            av1(k)
            if k < 6:
                qk23_step(k + 2)
        qk23_drain()

        # B2 with the panel-2 V projection interleaved, B3 with panel-3's:
        # keeps each attention group just-barely ACT-bound instead of
        # making B2 PE-bound while B3's tensor engine starves.
        v2_step, v2_drain = v_ebs(2)
        v3_step, v3_drain = v_ebs(3)

        def run_group(g, sx, ex, av, nk, vsched):
            sx(0)
            sx(1)
            sx(2)
            for k in range(nk):
                sx(k + 3)
                ex(k)
                step = vsched.get(k)
                if step is not None:
                    fn, arg = step
                    if fn is None:
                        arg()
                    else:
                        for eb in arg:
                            fn(eb)
                av(k)

        v2_sched = {0: (v2_step, (0, 1)), 1: (v2_step, (2, 3)),
                    2: (v2_step, (4, 5)), 3: (v2_step, (6, 7)),
                    4: (None, v2_drain)}
        sx2, ex2, av2, nk2, fin2 = attn_group(2)
        run_group(2, sx2, ex2, av2, nk2, v2_sched)

        v3_sched = {0: (v3_step, (0, 1)), 1: (v3_step, (2, 3)),
                    2: (v3_step, (4, 5)), 3: (v3_step, (6, 7)),
                    4: (None, v3_drain)}
        sx3, ex3, av3, nk3, fin3 = attn_group(3, defer_retire=True)
        run_group(3, sx3, ex3, av3, nk3, v3_sched)
        fin3()


def build_program():
    nc = bacc.Bacc("TRN2", target_bir_lowering=False, debug=False)
    x = nc.dram_tensor("x", [T, E], BF16, kind="ExternalInput").ap()
    wbt = nc.dram_tensor("wbt", [3 * H, E], BF16, kind="ExternalInput").ap()
    cbt_d = nc.dram_tensor("cbt_d", [CBC, 128], BF16, kind="ExternalInput").ap()
    out = nc.dram_tensor("out", [T, H], F32, kind="ExternalOutput").ap()
    with tile.TileContext(nc) as tc:
        _emit(nc, tc, x, wbt, cbt_d, out)
    nc.compile()
    return nc


_program = None


def _get_program():
    global _program
    if _program is None:
        _program = build_program()
    return _program


def make_in_maps(x, Wq, Wk, Wv, bq, bk, bv):
    bf = ml_dtypes.bfloat16
    x = np.asarray(x, np.float32).astype(bf)
    wall = np.concatenate(
        [np.asarray(w, np.float32) for w in (Wq, Wk, Wv)], axis=1
    ).astype(bf)
    wbt = np.ascontiguousarray(wall.T)
    cb = np.zeros((128, CBC), dtype=bf)
    cb[:, 0] = bf(1.0)
    cb[:, 1:129] = np.triu(np.ones((128, 128), np.float32)).astype(bf)
    cb[0, 129:257] = np.asarray(bv, np.float32).reshape(H).astype(bf)
    cb[:, 257] = np.asarray(bq, np.float32).reshape(H).astype(bf)
    cb[:, 258] = np.asarray(bk, np.float32).reshape(H).astype(bf)
    cbt_d = np.ascontiguousarray(cb.T)
    return [
        {"x": np.ascontiguousarray(x[b]), "wbt": wbt, "cbt_d": cbt_d}
        for b in range(B)
    ]


def kernel(x, Wq, Wk, Wv, bq, bk, bv):
    nc = _get_program()
    in_maps = make_in_maps(x, Wq, Wk, Wv, bq, bk, bv)
    res = run_bass_kernel_spmd(nc, in_maps, list(range(B)))
    return np.stack([res.results[b]["out"] for b in range(B)], axis=0).astype(
        np.float32
    )
